# revision 9
# baseline (speedup 1.0000x reference)
"""DSNN (spiking neural net) Trainium2 kernel.

reference semantics (per sample b, hidden unit h):
    hs = einsum('btd,hd->bth', inputs, weights)
    syn_t = ALPHA*syn_{t-1} + hs_t
    u_t   = BETA*mem~_{t-1} + syn_t          (pre-reset membrane)
    spk_t = (u_t > THR)
    mem~_t = u_t if u_t <= THR else 0        (hard reset, stored)
returns (spk, (mem_final, syn_final), (mem_seq, syn_seq))

Distribution: batch 64 -> 8 cores x 8 samples (embarrassingly parallel);
weights replicated.

Per-core dataflow, T in 8 chunks of 125:
  in:   DMA x [t=125p, (b,d)] (contiguous 2KB lines)
        PE-transpose 125x128 blocks -> xT [d=128p, (db,b,t)] (PSUM->SBUF via ACT)
  gemm: PE matmul K=512 (4 accumulating matmuls) -> hs PSUM [h=128p, 500]
        ACT copy -> SBUF bounce
  syn:  tensor_tensor_scan on Pool along t (alpha linear scan)
  mem:  sequential scan, 2 scalar_tensor_tensor DVE instrs per t on strided
        column APs [128, 8, 4] covering all (b, hblk):
           u = (mem~ * BETA) + syn ; mem~ = (u <= THR) * u
  spk:  (u > THR) in-place on Pool
  out:  PE-transpose each [h=128p, t=125] tile -> PSUM [t=125p, h=512],
        copy PSUM->SBUF staging (split DVE/ACT), DMA staging -> DRAM
        [b, t, h] with 2KB contiguous descriptors.
SBUF chunk buffers use free layout (b, t, hb), hb innermost.
"""

import numpy as np

ALPHA = 0.9
BETA = 0.85
THR = 0.1

B, T, DIN, H = 64, 1000, 512, 512
NCORES = 8
BS = B // NCORES          # batch shard per core = 8
TC = 125                  # t chunk
NCHUNK = T // TC          # 8
NHB = H // 128            # 4 h blocks
NDB = DIN // 128          # 4 d blocks

_cache = {}


def _build():
    import concourse.mybir as mybir
    import concourse.tile as tile
    from concourse import bacc
    from concourse.masks import make_identity

    f32 = mybir.dt.float32
    Alu = mybir.AluOpType

    nc = bacc.Bacc(None, target_bir_lowering=False)
    x_d = nc.dram_tensor("x", [BS, T, DIN], f32, kind="ExternalInput")
    wt_d = nc.dram_tensor("wt", [DIN, H], f32, kind="ExternalInput")  # W.T
    spk_d = nc.dram_tensor("spk", [BS, T, H], f32, kind="ExternalOutput")
    mem_d = nc.dram_tensor("mem", [BS, T, H], f32, kind="ExternalOutput")
    syn_d = nc.dram_tensor("syn", [BS, T, H], f32, kind="ExternalOutput")

    CH = BS * NHB * TC  # free size of one chunk buffer = 4000

    with tile.TileContext(nc) as tc:
        with (
            tc.tile_pool(name="consts", bufs=1) as consts,
            tc.tile_pool(name="xpool", bufs=2) as xpool,
            tc.tile_pool(name="xtpool", bufs=2) as xtpool,
            tc.tile_pool(name="hspool", bufs=4) as hspool,
            tc.tile_pool(name="synpool", bufs=2) as synpool,
            tc.tile_pool(name="upool", bufs=2) as upool,
            tc.tile_pool(name="mempool", bufs=2) as mempool,
            tc.tile_pool(name="stgpool", bufs=6) as stgpool,
            tc.tile_pool(name="tppsum", bufs=2, space="PSUM") as tppsum,
            tc.tile_pool(name="mmpsum", bufs=2, space="PSUM") as mmpsum,
            tc.tile_pool(name="otpsum", bufs=4, space="PSUM") as otpsum,
        ):
            # ---- constants ----
            wts = []
            for db in range(NDB):
                w = consts.tile([128, H], f32, tag=f"wt{db}")
                nc.sync.dma_start(out=w[:, :], in_=wt_d[db * 128:(db + 1) * 128, :])
                wts.append(w)
            ident = consts.tile([128, 128], f32, tag="ident")
            make_identity(nc, ident[:, :])
            alpha_t = consts.tile([128, TC], f32, tag="alpha")
            nc.gpsimd.memset(alpha_t[:, :], ALPHA)
            zst = consts.tile([128, BS * NHB], f32, tag="zstate")
            nc.gpsimd.memset(zst[:, :], 0.0)
            zst3 = zst[:, :].rearrange("p (b h) -> p b h", b=BS)

            prev_syn3 = None
            prev_mem3 = None
            ncopy = 0  # round-robin out-copy engine assignment

            for c in range(NCHUNK):
                t0 = c * TC

                # ---- DMA in x chunk: [125p(t), b, d] ----
                xb = xpool.tile([128, BS * DIN], f32, tag="x")
                xv = xb[:, :].rearrange("p (b d) -> p b d", b=BS)
                for half in range(2):
                    bsl = slice(half * 4, half * 4 + 4)
                    nc.sync.dma_start(
                        out=xv[:TC, bsl, :],
                        in_=x_d[bsl, t0:t0 + TC, :].rearrange("b t d -> t b d"),
                    )

                # ---- transpose x -> xT [d=128p, (db, b, t)] ----
                xt = xtpool.tile([128, NDB * BS * TC], f32, tag="xt")
                for db in range(NDB):
                    for half in range(2):
                        pt = tppsum.tile([128, 4 * TC], f32, tag="tp")
                        for bi in range(4):
                            b = half * 4 + bi
                            nc.tensor.transpose(
                                pt[:, bi * TC:(bi + 1) * TC],
                                xv[:TC, b, db * 128:(db + 1) * 128],
                                ident[:TC, :TC],
                            )
                        nc.scalar.copy(
                            out=xt[:, db * BS * TC + half * 4 * TC:
                                   db * BS * TC + (half * 4 + 4) * TC],
                            in_=pt[:, :],
                        )

                # ---- chunk buffers: free layout (b, t, hb), hb innermost ----
                syn_b = synpool.tile([128, CH], f32, tag="syn")
                u_b = upool.tile([128, CH], f32, tag="u")
                mem_b = mempool.tile([128, CH], f32, tag="mem")
                syn3 = syn_b[:, :].rearrange("p (b t h) -> p b t h", b=BS, t=TC)
                u3 = u_b[:, :].rearrange("p (b t h) -> p b t h", b=BS, t=TC)
                mem3 = mem_b[:, :].rearrange("p (b t h) -> p b t h", b=BS, t=TC)

                # ---- GEMM -> hs bounce -> alpha-scan into syn (Pool) ----
                for hb in range(NHB):
                    for half in range(2):
                        pm = mmpsum.tile([128, 4 * TC], f32, tag="mm")
                        for db in range(NDB):
                            nc.tensor.matmul(
                                pm[:, :],
                                lhsT=wts[db][:, hb * 128:(hb + 1) * 128],
                                rhs=xt[:, db * BS * TC + half * 4 * TC:
                                       db * BS * TC + (half * 4 + 4) * TC],
                                start=(db == 0),
                                stop=(db == NDB - 1),
                            )
                        hsb = hspool.tile([128, 4 * TC], f32, tag="hs")
                        nc.scalar.copy(out=hsb[:, :], in_=pm[:, :])
                        for bi in range(4):
                            b = half * 4 + bi
                            init = (
                                0.0 if c == 0
                                else prev_syn3[:, b, TC - 1:TC, hb]
                            )
                            nc.vector.tensor_tensor_scan(
                                out=syn3[:, b, :, hb],
                                data0=alpha_t[:, :],
                                data1=hsb[:, bi * TC:(bi + 1) * TC],
                                initial=init,
                                op0=Alu.mult,
                                op1=Alu.add,
                            )

                # ---- sequential mem scan (2 DVE instrs / step) ----
                for t in range(TC):
                    if t == 0:
                        mprev = zst3 if c == 0 else prev_mem3[:, :, TC - 1, :]
                    else:
                        mprev = mem3[:, :, t - 1, :]
                    nc.vector.scalar_tensor_tensor(
                        out=u3[:, :, t, :],
                        in0=mprev,
                        scalar=BETA,
                        in1=syn3[:, :, t, :],
                        op0=Alu.mult,
                        op1=Alu.add,
                    )
                    nc.vector.scalar_tensor_tensor(
                        out=mem3[:, :, t, :],
                        in0=u3[:, :, t, :],
                        scalar=THR,
                        in1=u3[:, :, t, :],
                        op0=Alu.is_le,
                        op1=Alu.mult,
                    )

                # ---- spikes: u > THR, in place (Pool) ----
                nc.gpsimd.tensor_scalar(
                    u_b[:, :], u_b[:, :], THR, None, Alu.is_gt,
                )

                # ---- out: transpose [h,t] -> [t,h], copy to staging, DMA ----
                for b in range(BS):
                    for buf3, dram in ((syn3, syn_d), (mem3, mem_d), (u3, spk_d)):
                        po = otpsum.tile([128, H], f32, tag="ot")
                        for hb in range(NHB):
                            nc.tensor.transpose(
                                po[:TC, hb * 128:(hb + 1) * 128],
                                buf3[:, b, :, hb],
                                ident[:, :],
                            )
                        stg = stgpool.tile([128, H], f32, tag="stg")
                        if ncopy % 3 == 0:
                            nc.vector.tensor_copy(stg[:TC, :], po[:TC, :])
                        else:
                            nc.scalar.copy(out=stg[:TC, :], in_=po[:TC, :])
                        ncopy += 1
                        nc.sync.dma_start(
                            out=dram[b, t0:t0 + TC, :],
                            in_=stg[:TC, :],
                        )

                prev_syn3 = syn3
                prev_mem3 = mem3

    nc.compile()
    return nc


def _get_nc():
    if "nc" not in _cache:
        _cache["nc"] = _build()
    return _cache["nc"]


def kernel(inputs: np.ndarray, weights: np.ndarray):
    from concourse.bass_utils import run_bass_kernel_spmd

    nc = _get_nc()
    inputs = np.ascontiguousarray(inputs, dtype=np.float32)
    wt = np.ascontiguousarray(weights.astype(np.float32).T)
    in_maps = [
        {"x": np.ascontiguousarray(inputs[i * BS:(i + 1) * BS]), "wt": wt}
        for i in range(NCORES)
    ]
    res = run_bass_kernel_spmd(nc, in_maps, core_ids=list(range(NCORES)))
    _cache["last_result"] = res
    spk = np.concatenate([res.results[i]["spk"] for i in range(NCORES)], axis=0)
    mem_seq = np.concatenate([res.results[i]["mem"] for i in range(NCORES)], axis=0)
    syn_seq = np.concatenate([res.results[i]["syn"] for i in range(NCORES)], axis=0)
    mem_f = np.ascontiguousarray(mem_seq[:, -1])
    syn_f = np.ascontiguousarray(syn_seq[:, -1])
    return (spk, (mem_f, syn_f), (mem_seq, syn_seq))


# revision 10
# speedup vs baseline: 1.6736x; 1.6736x over previous
"""DSNN (spiking neural net) Trainium2 kernel.

reference semantics (per sample b, hidden unit h):
    hs = einsum('btd,hd->bth', inputs, weights)
    syn_t = ALPHA*syn_{t-1} + hs_t
    u_t   = BETA*mem~_{t-1} + syn_t          (pre-reset membrane)
    spk_t = (u_t > THR)
    mem~_t = u_t if u_t <= THR else 0        (hard reset, stored)
returns (spk, (mem_final, syn_final), (mem_seq, syn_seq))

Distribution: batch 64 -> 8 cores x 8 samples (embarrassingly parallel);
weights replicated.

Per-core dataflow, T in 8 chunks of 125:
  in:   DMA x [t=125p, (b,d)] (contiguous 2KB lines)
        PE-transpose 125x128 blocks -> xT [d=128p, (db,b,t)] (PSUM->SBUF via ACT)
  gemm: PE matmul K=512 (4 accumulating matmuls) -> hs PSUM [h=128p, 500]
        ACT copy -> SBUF bounce
  syn:  tensor_tensor_scan on Pool along t (alpha linear scan)
  mem:  sequential scan, 2 scalar_tensor_tensor DVE instrs per t on strided
        column APs [128, 8, 4] covering all (b, hblk):
           u = (mem~ * BETA) + syn ; mem~ = (u <= THR) * u
  spk:  (u > THR) in-place on Pool
  out:  PE-transpose each [h=128p, t=125] tile -> PSUM [t=125p, h=512],
        copy PSUM->SBUF staging (split DVE/ACT), DMA staging -> DRAM
        [b, t, h] with 2KB contiguous descriptors.
SBUF chunk buffers use free layout (b, t, hb), hb innermost.
"""

import numpy as np

ALPHA = 0.9
BETA = 0.85
THR = 0.1

B, T, DIN, H = 64, 1000, 512, 512
NCORES = 8
BS = B // NCORES          # batch shard per core = 8
TC = 125                  # t chunk
NCHUNK = T // TC          # 8
NHB = H // 128            # 4 h blocks
NDB = DIN // 128          # 4 d blocks

_cache = {}


def _build():
    import concourse.mybir as mybir
    import concourse.tile as tile
    from concourse import bacc
    from concourse.masks import make_identity

    f32 = mybir.dt.float32
    Alu = mybir.AluOpType

    nc = bacc.Bacc(None, target_bir_lowering=False)
    x_d = nc.dram_tensor("x", [BS, T, DIN], f32, kind="ExternalInput")
    wt_d = nc.dram_tensor("wt", [DIN, H], f32, kind="ExternalInput")  # W.T
    spk_d = nc.dram_tensor("spk", [BS, T, H], f32, kind="ExternalOutput")
    mem_d = nc.dram_tensor("mem", [BS, T, H], f32, kind="ExternalOutput")
    syn_d = nc.dram_tensor("syn", [BS, T, H], f32, kind="ExternalOutput")

    CH = BS * NHB * TC  # free size of one chunk buffer = 4000

    with tile.TileContext(nc) as tc:
        with (
            tc.tile_pool(name="consts", bufs=1) as consts,
            tc.tile_pool(name="xpool", bufs=2) as xpool,
            tc.tile_pool(name="xtpool", bufs=2) as xtpool,
            tc.tile_pool(name="hspool", bufs=4) as hspool,
            tc.tile_pool(name="synpool", bufs=2) as synpool,
            tc.tile_pool(name="upool", bufs=2) as upool,
            tc.tile_pool(name="mempool", bufs=2) as mempool,
            tc.tile_pool(name="stgpool", bufs=6) as stgpool,
            tc.tile_pool(name="tppsum", bufs=2, space="PSUM") as tppsum,
            tc.tile_pool(name="mmpsum", bufs=2, space="PSUM") as mmpsum,
            tc.tile_pool(name="otpsum", bufs=4, space="PSUM") as otpsum,
        ):
            # ---- constants ----
            wts = []
            for db in range(NDB):
                w = consts.tile([128, H], f32, tag=f"wt{db}")
                nc.sync.dma_start(out=w[:, :], in_=wt_d[db * 128:(db + 1) * 128, :])
                wts.append(w)
            ident = consts.tile([128, 128], f32, tag="ident")
            make_identity(nc, ident[:, :])
            alpha_t = consts.tile([128, TC], f32, tag="alpha")
            nc.gpsimd.memset(alpha_t[:, :], ALPHA)
            zst = consts.tile([128, BS * NHB], f32, tag="zstate")
            nc.gpsimd.memset(zst[:, :], 0.0)
            zst3 = zst[:, :].rearrange("p (b h) -> p b h", b=BS)

            prev_syn3 = None
            prev_mem3 = None
            ncopy = 0  # round-robin out-copy engine assignment

            for c in range(NCHUNK):
                t0 = c * TC

                # ---- DMA in x chunk: [125p(t), b, d] ----
                xb = xpool.tile([128, BS * DIN], f32, tag="x")
                xv = xb[:, :].rearrange("p (b d) -> p b d", b=BS)
                for half in range(2):
                    bsl = slice(half * 4, half * 4 + 4)
                    nc.sync.dma_start(
                        out=xv[:TC, bsl, :],
                        in_=x_d[bsl, t0:t0 + TC, :].rearrange("b t d -> t b d"),
                    )

                # ---- transpose x -> xT [d=128p, (db, b, t)] ----
                xt = xtpool.tile([128, NDB * BS * TC], f32, tag="xt")
                for db in range(NDB):
                    for half in range(2):
                        pt = tppsum.tile([128, 4 * TC], f32, tag="tp")
                        for bi in range(4):
                            b = half * 4 + bi
                            nc.tensor.transpose(
                                pt[:, bi * TC:(bi + 1) * TC],
                                xv[:TC, b, db * 128:(db + 1) * 128],
                                ident[:TC, :TC],
                            )
                        nc.scalar.copy(
                            out=xt[:, db * BS * TC + half * 4 * TC:
                                   db * BS * TC + (half * 4 + 4) * TC],
                            in_=pt[:, :],
                        )

                # ---- chunk buffers: free layout (b, t, hb), hb innermost ----
                syn_b = synpool.tile([128, CH], f32, tag="syn")
                u_b = upool.tile([128, CH], f32, tag="u")
                mem_b = mempool.tile([128, CH], f32, tag="mem")
                syn3 = syn_b[:, :].rearrange("p (t b h) -> p t b h", t=TC, b=BS)
                u3 = u_b[:, :].rearrange("p (t b h) -> p t b h", t=TC, b=BS)
                mem3 = mem_b[:, :].rearrange("p (t b h) -> p t b h", t=TC, b=BS)

                # ---- GEMM -> hs bounce -> alpha-scan into syn (Pool) ----
                for hb in range(NHB):
                    for half in range(2):
                        pm = mmpsum.tile([128, 4 * TC], f32, tag="mm")
                        for db in range(NDB):
                            nc.tensor.matmul(
                                pm[:, :],
                                lhsT=wts[db][:, hb * 128:(hb + 1) * 128],
                                rhs=xt[:, db * BS * TC + half * 4 * TC:
                                       db * BS * TC + (half * 4 + 4) * TC],
                                start=(db == 0),
                                stop=(db == NDB - 1),
                            )
                        hsb = hspool.tile([128, 4 * TC], f32, tag="hs")
                        nc.scalar.copy(out=hsb[:, :], in_=pm[:, :])
                        for bi in range(4):
                            b = half * 4 + bi
                            init = (
                                0.0 if c == 0
                                else prev_syn3[:, TC - 1:TC, b, hb]
                            )
                            nc.vector.tensor_tensor_scan(
                                out=syn3[:, :, b, hb],
                                data0=alpha_t[:, :],
                                data1=hsb[:, bi * TC:(bi + 1) * TC],
                                initial=init,
                                op0=Alu.mult,
                                op1=Alu.add,
                            )

                # ---- sequential mem scan (2 DVE instrs / step) ----
                for t in range(TC):
                    if t == 0:
                        mprev = zst3 if c == 0 else prev_mem3[:, TC - 1, :, :]
                    else:
                        mprev = mem3[:, t - 1, :, :]
                    nc.vector.scalar_tensor_tensor(
                        out=u3[:, t, :, :],
                        in0=mprev,
                        scalar=BETA,
                        in1=syn3[:, t, :, :],
                        op0=Alu.mult,
                        op1=Alu.add,
                    )
                    nc.vector.scalar_tensor_tensor(
                        out=mem3[:, t, :, :],
                        in0=u3[:, t, :, :],
                        scalar=THR,
                        in1=u3[:, t, :, :],
                        op0=Alu.is_le,
                        op1=Alu.mult,
                    )

                # ---- spikes: u > THR, in place (DVE; Pool is ~15ns/elem here) ----
                nc.vector.tensor_scalar(
                    u_b[:, :], u_b[:, :], THR, None, Alu.is_gt,
                )

                # ---- out: transpose [h,t] -> [t,h], copy to staging, DMA ----
                for b in range(BS):
                    for buf3, dram in ((syn3, syn_d), (mem3, mem_d), (u3, spk_d)):
                        po = otpsum.tile([128, H], f32, tag="ot")
                        for hb in range(NHB):
                            nc.tensor.transpose(
                                po[:TC, hb * 128:(hb + 1) * 128],
                                buf3[:, :, b, hb],
                                ident[:, :],
                            )
                        stg = stgpool.tile([128, H], f32, tag="stg")
                        nc.scalar.copy(out=stg[:TC, :], in_=po[:TC, :])
                        ncopy += 1
                        nc.sync.dma_start(
                            out=dram[b, t0:t0 + TC, :],
                            in_=stg[:TC, :],
                        )

                prev_syn3 = syn3
                prev_mem3 = mem3

    nc.compile()
    return nc


def _get_nc():
    if "nc" not in _cache:
        _cache["nc"] = _build()
    return _cache["nc"]


def kernel(inputs: np.ndarray, weights: np.ndarray):
    from concourse.bass_utils import run_bass_kernel_spmd

    nc = _get_nc()
    inputs = np.ascontiguousarray(inputs, dtype=np.float32)
    wt = np.ascontiguousarray(weights.astype(np.float32).T)
    in_maps = [
        {"x": np.ascontiguousarray(inputs[i * BS:(i + 1) * BS]), "wt": wt}
        for i in range(NCORES)
    ]
    res = run_bass_kernel_spmd(nc, in_maps, core_ids=list(range(NCORES)))
    _cache["last_result"] = res
    spk = np.concatenate([res.results[i]["spk"] for i in range(NCORES)], axis=0)
    mem_seq = np.concatenate([res.results[i]["mem"] for i in range(NCORES)], axis=0)
    syn_seq = np.concatenate([res.results[i]["syn"] for i in range(NCORES)], axis=0)
    mem_f = np.ascontiguousarray(mem_seq[:, -1])
    syn_f = np.ascontiguousarray(syn_seq[:, -1])
    return (spk, (mem_f, syn_f), (mem_seq, syn_seq))
